# revision 7
# baseline (speedup 1.0000x reference)
"""Trainium2 Bass kernel for nn_Brep_Gcn (GCN message passing).

Math (reference):
    x  = relu(sum_ch conv1d(feature))           # conv folds to a banded matmul
    h  = relu(A @ (x W1) + b1) = relu((A @ x) W1 + b1)
    y  = A @ (h W2) + b2

Reordered for memory efficiency:
    S1 = A @ x          (sparse gather + segment-sum, fp16 rows)
    h  = relu(S1 W1 + b1)
    P  = h W2
    y  = A @ P + b2     (sparse gather + segment-sum, fp16 rows)

Distribution: nodes row-sharded across 8 cores; edges partitioned by
destination owner.  The conv prelude is REPLICATED on every core (writing a
private fp16 x_full) which removes the x AllGather entirely; P is exchanged
with one AllGather.  Weights replicated.

Sparse segment-sum on device: edges sorted by (dest-window, src-chunk),
padded to 128-edge blocks.  Per block: dma_gather the 128 source rows (fp16,
256 B rows), build a one-hot selector Sel[e, d] = val[e] * (slot[e] == d)
with one fused tensor_scalar, and matmul on the PE accumulating into PSUM
per (window, chunk) segment.  Gather calls batch 32 blocks (4096 idxs) and
round-robin the 4 SWDGE queues so descriptor generation runs on all four
Q7 core-pairs concurrently.
"""

import math
import os
import sys
from dataclasses import dataclass

import numpy as np

sys.path.insert(0, "/opt/trn_rl_repo")

import concourse.bass as bass
import concourse.tile as tile
from concourse import bacc
from concourse import mybir
from concourse.bass_utils import run_bass_kernel_spmd
from concourse.masks import make_identity

F32 = mybir.dt.float32
F16 = mybir.dt.float16
I16 = mybir.dt.int16
I32 = mybir.dt.int32
AF = mybir.ActivationFunctionType
OP = mybir.AluOpType


@dataclass
class Cfg:
    N: int = 100000
    E: int = 3200000
    D_IN: int = 83
    D_HID: int = 1024
    NCLS: int = 25
    NCORES: int = 8
    NCHUNK: int = 4          # source-index chunks (int16 gather indices)
    XPAD: int = 128          # padded x row, fp16 (256 B)
    PPAD: int = 128          # padded P row, fp16 (256 B)
    CPAD: int = 32           # classes padded (25 -> 32)
    GBLK: int = 32           # 128-edge blocks per dma_gather call (4096 idxs)
    IDXG: int = 4            # gather calls per idx-staging DMA
    NQ: int = 4              # SWDGE queues (round-robin)

    @property
    def NSH(self):
        return self.N // self.NCORES

    @property
    def CHUNK(self):
        return self.N // self.NCHUNK

    @property
    def NW(self):            # dest windows (of 128) per core
        return (self.NSH + 127) // 128

    @property
    def NJ(self):            # hidden dim in 128-blocks
        return self.D_HID // 128


# ----------------------------------------------------------------------------
# Host-side preprocessing
# ----------------------------------------------------------------------------

def _wrap_idx16(idx: np.ndarray) -> np.ndarray:
    """dma_gather index layout: idx i at [i % 16, i // 16], tiled to 128
    partitions (replicated for the 8 Q7 cores)."""
    assert idx.size % 16 == 0
    a = idx.reshape(-1, 16).T.astype(np.int16)       # [16, n/16]
    return np.tile(a, (8, 1))                        # [128, n/16]


def build_host(cfg: Cfg, inputs: dict) -> tuple[list[dict], dict]:
    """Returns (per-core input maps, shared structure metadata)."""
    N, E = cfg.N, cfg.E
    NSH, NW, NCH, CH = cfg.NSH, cfg.NW, cfg.NCHUNK, cfg.CHUNK

    feature = np.asarray(inputs["feature"], np.float32)
    conv_w = np.asarray(inputs["conv_w"], np.float32)
    conv_b = np.asarray(inputs["conv_b"], np.float32)
    W1 = np.asarray(inputs["W1"], np.float32)
    b1 = np.asarray(inputs["b1"], np.float32)
    W2 = np.asarray(inputs["W2"], np.float32)
    b2 = np.asarray(inputs["b2"], np.float32)
    val = np.asarray(inputs["adj_val"], np.float32)
    row = np.asarray(inputs["edge_row"], np.int64)
    col = np.asarray(inputs["edge_col"], np.int64)

    # conv1d(1->4, k=5, pad 2) summed over channels == banded matmul.
    ws = conv_w.sum(axis=0).ravel()                  # [5]
    b0 = float(conv_b.sum())
    C = np.zeros((cfg.D_IN, cfg.XPAD), np.float32)
    for i in range(cfg.D_IN):
        for k in range(5):
            j = i - (k - 2)                          # out[:, j] += ws[k] * in[:, j + k - 2]
            if 0 <= j < cfg.D_IN:
                C[i, j] = ws[k]

    # ---- edge partitioning: by dest core, then (dest-window, src-chunk) ----
    core_of = row // NSH
    per_core = []
    cnt = np.zeros((cfg.NCORES, NW, NCH), np.int64)
    for k in range(cfg.NCORES):
        m = core_of == k
        r, c_, v = row[m] - k * NSH, col[m], val[m]
        w = r >> 7
        ch = c_ // CH
        order = np.lexsort((c_, ch, w))
        r, c_, v, w, ch = r[order], c_[order], v[order], w[order], ch[order]
        key = w * NCH + ch
        cnt[k] = np.bincount(key, minlength=NW * NCH).reshape(NW, NCH)
        per_core.append((r, c_, v, key))

    # uniform block counts across cores (same program on every core)
    M = np.maximum(1, np.ceil(cnt.max(axis=0) / 128).astype(np.int64))  # [NW, NCH]

    # block metadata, chunk-major (same for every core)
    blocks = []      # (w, chunk, seg_first, seg_last)
    calls = []       # (chunk, blk_start, nblk, idx_off16)
    nblk_total = int(M.sum())
    for ch in range(NCH):
        cblks = []
        for w in range(NW):
            for m in range(int(M[w, ch])):
                cblks.append((w, ch, m == 0, m == int(M[w, ch]) - 1))
        s = 0
        while s < len(cblks):
            n = min(cfg.GBLK, len(cblks) - s)
            calls.append([ch, len(blocks) + s, n, 0])
            s += n
        blocks.extend(cblks)
    assert len(blocks) == nblk_total
    off = 0
    for call in calls:
        call[3] = off
        off += call[2] * 128 // 16
    tot16 = off

    # ---- per-core padded edge arrays in block order ----
    in_maps = []
    for k in range(cfg.NCORES):
        r, c_, v, key = per_core[k]
        pos = np.searchsorted(key, np.arange(NW * NCH + 1), side="left")
        idx_pad = np.zeros(nblk_total * 128, np.int16)
        slot_pad = np.zeros(nblk_total * 128, np.float32)
        val_pad = np.zeros(nblk_total * 128, np.float32)
        bi = 0
        for ch in range(NCH):
            for w in range(NW):
                a, b = pos[w * NCH + ch], pos[w * NCH + ch + 1]
                n = b - a
                mb = int(M[w, ch])
                dst = bi * 128
                idx_pad[dst:dst + n] = (c_[a:b] % CH).astype(np.int16)
                slot_pad[dst:dst + n] = (r[a:b] - (w << 7)).astype(np.float32)
                val_pad[dst:dst + n] = v[a:b]
                bi += mb
        assert bi == nblk_total
        idx_arr = np.zeros((128, tot16), np.int16)
        for ch, bs, nb, o16 in calls:
            seg = idx_pad[bs * 128:(bs + nb) * 128]
            idx_arr[:, o16:o16 + nb * 128 // 16] = _wrap_idx16(seg)
        slot_arr = slot_pad.reshape(nblk_total, 128).T.copy()
        val_arr = val_pad.reshape(nblk_total, 128).T.copy()

        b1c = b1.reshape(cfg.NJ, 128).T.astype(np.float32).copy()  # [128, NJ]
        W2p = np.zeros((cfg.D_HID, cfg.CPAD), np.float16)
        W2p[:, :cfg.NCLS] = W2.astype(np.float16)
        b2c = np.zeros((cfg.CPAD, 1), np.float32)
        b2c[:cfg.NCLS, 0] = b2

        in_maps.append({
            "feature": feature,
            "Cmat": C.astype(np.float16),
            "W1": W1.astype(np.float16),
            "b1c": b1c,
            "W2p": W2p,
            "b2c": b2c,
            "idx_dr": idx_arr,
            "slot_dr": slot_arr,
            "val_dr": val_arr,
        })

    meta = {"blocks": blocks, "calls": calls, "nblk": nblk_total,
            "tot16": tot16, "b0": b0}
    return in_maps, meta


# ----------------------------------------------------------------------------
# Bass program (identical for every core; per-core data comes via inputs)
# ----------------------------------------------------------------------------

def build_program(cfg: Cfg, meta: dict) -> bass.Bass:
    N, NSH, NW, NCH, CH = cfg.N, cfg.NSH, cfg.NW, cfg.NCHUNK, cfg.CHUNK
    NJ, XP, PP, CP = cfg.NJ, cfg.XPAD, cfg.PPAD, cfg.CPAD
    blocks, calls = meta["blocks"], meta["calls"]
    nblk, tot16 = meta["nblk"], meta["tot16"]
    groups = [list(range(cfg.NCORES))]

    nc = bacc.Bacc("TRN2", target_bir_lowering=False, debug=False,
                   num_devices=cfg.NCORES, num_swdge_queues=cfg.NQ)

    feature = nc.declare_dram_parameter("feature", [N, cfg.D_IN], F32, isOutput=False)
    Cmat = nc.declare_dram_parameter("Cmat", [cfg.D_IN, XP], F16, isOutput=False)
    W1 = nc.declare_dram_parameter("W1", [cfg.D_IN, cfg.D_HID], F16, isOutput=False)
    b1c = nc.declare_dram_parameter("b1c", [128, NJ], F32, isOutput=False)
    W2p = nc.declare_dram_parameter("W2p", [cfg.D_HID, CP], F16, isOutput=False)
    b2c = nc.declare_dram_parameter("b2c", [CP, 1], F32, isOutput=False)
    idx_dr = nc.declare_dram_parameter("idx_dr", [128, tot16], I16, isOutput=False)
    slot_dr = nc.declare_dram_parameter("slot_dr", [128, nblk], F32, isOutput=False)
    val_dr = nc.declare_dram_parameter("val_dr", [128, nblk], F32, isOutput=False)
    logits = nc.declare_dram_parameter("logits", [NSH, cfg.NCLS], F32, isOutput=True)

    x_full = nc.dram_tensor("x_full", [N, XP], F16)
    p_sh = nc.dram_tensor("p_sh", [NSH, PP], F16)
    p_full = nc.dram_tensor("p_full", [N, PP], F16, addr_space="Shared")

    with tile.TileContext(nc) as tc:
        with (
            tc.tile_pool(name="singles", bufs=1) as singles,
            tc.tile_pool(name="work", bufs=3) as work,
            tc.tile_pool(name="cvin", bufs=2) as cvin,
            tc.tile_pool(name="cvout", bufs=2) as cvout,
            tc.tile_pool(name="sel", bufs=8) as selp,
            tc.tile_pool(name="gath", bufs=3) as gathp,
            tc.tile_pool(name="ht", bufs=18) as htp,
            tc.tile_pool(name="psA", bufs=4, space="PSUM") as psA,
            tc.tile_pool(name="psSeg", bufs=2, space="PSUM") as psSeg,
            tc.tile_pool(name="psP", bufs=1, space="PSUM") as psP,
            tc.tile_pool(name="psT2", bufs=1, space="PSUM") as psT2,
        ):
            # ---------------- constants ----------------
            C_sb = singles.tile([cfg.D_IN, XP], F16)
            nc.sync.dma_start(out=C_sb[:], in_=Cmat[:])
            W1_sb = singles.tile([cfg.D_IN, cfg.D_HID], F16)
            nc.sync.dma_start(out=W1_sb[:], in_=W1[:])
            b1_sb = singles.tile([128, NJ], F32)
            nc.sync.dma_start(out=b1_sb[:], in_=b1c[:])
            W2_sb = singles.tile([128, NJ, CP], F16)
            nc.sync.dma_start(out=W2_sb[:], in_=W2p.rearrange("(j p) q -> p j q", p=128))
            b2_sb = singles.tile([CP, 1], F32)
            nc.sync.dma_start(out=b2_sb[:], in_=b2c[:])
            slot_sb = singles.tile([128, nblk], F32)
            nc.sync.dma_start(out=slot_sb[:], in_=slot_dr[:])
            val_sb = singles.tile([128, nblk], F32)
            nc.sync.dma_start(out=val_sb[:], in_=val_dr[:])

            b0_sb = singles.tile([128, 1], F32)
            nc.vector.memset(b0_sb[:], meta["b0"])
            ident = singles.tile([128, 128], F32)
            make_identity(nc, ident[:])
            ident16 = singles.tile([128, 128], F16)
            nc.vector.tensor_copy(out=ident16[:], in_=ident[:])
            iota_i = singles.tile([128, 128], I32)
            nc.gpsimd.iota(iota_i[:], pattern=[[1, 128]], base=0, channel_multiplier=0)
            iota16 = singles.tile([128, 128], F16)
            nc.vector.tensor_copy(out=iota16[:], in_=iota_i[:])

            S1T = singles.tile([cfg.D_IN, NW * 128], F32)
            nc.vector.memset(S1T[:], 0.0)
            # logitT init = b2 broadcast along the free dim
            logitT = singles.tile([CP, NW * 128], F32)
            b2_ap = b2_sb[:]
            b2_bc = bass.AP(tensor=b2_ap.tensor, offset=b2_ap.offset,
                            ap=[b2_ap.ap[0], [0, NW * 128]])
            nc.vector.tensor_copy(out=logitT[:], in_=b2_bc)
            logit_sb = singles.tile([128, NW, CP], F32)

            # ---------------- phase A: replicated conv -> x_full (fp16) -----
            CVB = 4                       # 128-row subtiles per load batch
            nfull = N // 128              # 781 full tiles (+ 32-row tail)
            a_tiles = []                  # (row0, rows) per batch
            t = 0
            while t + CVB <= nfull:
                a_tiles.append((t * 128, CVB * 128))
                t += CVB
            while t < nfull:
                a_tiles.append((t * 128, 128))
                t += 1
            if N % 128:
                a_tiles.append((nfull * 128, N % 128))

            for (r0, rows) in a_tiles:
                nb4 = (rows + 127) // 128
                if rows % 128 == 0 and nb4 > 1:
                    ftb = cvin.tile([128, CVB, cfg.D_IN], F32, tag="ftb")
                    nc.sync.dma_start(
                        out=ftb[:, :nb4, :],
                        in_=feature[r0:r0 + rows].rearrange("(a p) d -> p a d", p=128))
                else:
                    ftb = cvin.tile([128, CVB, cfg.D_IN], F32, tag="ftb")
                    nc.sync.dma_start(out=ftb[:rows, 0, :],
                                      in_=feature[r0:r0 + rows])
                xtb = cvout.tile([128, CVB, XP], F16, tag="xtb")
                for a in range(nb4):
                    sub = min(128, rows - a * 128)
                    ps_t = psA.tile([128, 128], F32, tag="pa", name="ps_t")
                    nc.tensor.transpose(out=ps_t[:cfg.D_IN, :sub],
                                        in_=ftb[:sub, a, :],
                                        identity=ident[:sub, :sub])
                    ftT = work.tile([cfg.D_IN, 128], F16, tag="ftT")
                    nc.any.tensor_copy(out=ftT[:, :sub], in_=ps_t[:cfg.D_IN, :sub])
                    ps_x = psA.tile([128, XP], F32, tag="pa", name="ps_x")
                    nc.tensor.matmul(out=ps_x[:sub], lhsT=ftT[:, :sub], rhs=C_sb[:],
                                     start=True, stop=True)
                    nc.scalar.activation(out=xtb[:sub, a, :], in_=ps_x[:sub],
                                         func=AF.Relu, bias=b0_sb[:sub])
                if rows % 128 == 0 and nb4 > 1:
                    nc.sync.dma_start(
                        out=x_full[r0:r0 + rows].rearrange("(a p) d -> p a d", p=128),
                        in_=xtb[:, :nb4, :])
                else:
                    nc.sync.dma_start(out=x_full[r0:r0 + rows], in_=xtb[:rows, 0, :])

            # ---------------- phase B: L1 SpMM  S1T = (A @ x).T ----------------
            def spmm(src, src_elem, out_cb):
                """Shared gather + one-hot-matmul skeleton for phases B/D."""
                idx_t = None
                g0 = 0
                for ci, (ch, bs, nb, o16) in enumerate(calls):
                    if ci % cfg.IDXG == 0:
                        grp = calls[ci:ci + cfg.IDXG]
                        g0 = o16
                        gn = sum(c[2] for c in grp) * 8
                        idx_t = work.tile([128, cfg.GBLK * 8 * cfg.IDXG], I16,
                                          tag="idx")
                        nc.sync.dma_start(out=idx_t[:, :gn],
                                          in_=idx_dr[:, g0:g0 + gn])
                    n16 = nb * 128 // 16
                    gt = gathp.tile([128, cfg.GBLK, src_elem], F16, tag="gt")
                    nc.gpsimd.dma_gather(
                        out_ap=gt[:, :nb, :],
                        in_ap=src[ch * CH:(ch + 1) * CH, :],
                        idxs_ap=idx_t[:, o16 - g0:o16 - g0 + n16],
                        num_idxs=nb * 128, num_idxs_reg=nb * 128,
                        elem_size=src_elem, queue_num=ci % cfg.NQ,
                        single_packet=(nb * 128 <= 1024))
                    for j in range(nb):
                        B = bs + j
                        w, _ch, sf, sl = blocks[B]
                        sel = selp.tile([128, 128], F16, tag="sel")
                        nc.any.tensor_scalar(
                            out=sel[:], in0=iota16[:],
                            scalar1=slot_sb[:, B:B + 1],
                            scalar2=val_sb[:, B:B + 1],
                            op0=OP.is_equal, op1=OP.mult)
                        out_cb(w, sf, sl, gt, j, sel)

            ps_seg = [None]

            def b_block(w, sf, sl, gt, j, sel):
                wsize = min(128, NSH - w * 128)
                if sf:
                    ps_seg[0] = psSeg.tile([128, 128], F32, tag="seg", name="ps_seg")
                nc.tensor.matmul(out=ps_seg[0][:], lhsT=gt[:, j, :], rhs=sel[:],
                                 start=sf, stop=sl)
                if sl:
                    nc.vector.tensor_add(
                        out=S1T[:, w * 128:w * 128 + wsize],
                        in0=S1T[:, w * 128:w * 128 + wsize],
                        in1=ps_seg[0][:cfg.D_IN, :wsize])

            spmm(x_full, XP, b_block)

            # ---------------- phase C: h = relu(S1 W1 + b1); PT = (h W2).T ----
            for d in range(NW):
                wsize = min(128, NSH - d * 128)
                s1w = work.tile([cfg.D_IN, 128], F16, tag="s1w")
                nc.any.tensor_copy(out=s1w[:, :wsize],
                                   in_=S1T[:, d * 128:d * 128 + wsize])
                hts = []
                for j in range(NJ):
                    ps_h = psA.tile([128, 128], F32, tag="pa", name="ps_h")
                    nc.tensor.matmul(out=ps_h[:, :wsize],
                                     lhsT=W1_sb[:, j * 128:(j + 1) * 128],
                                     rhs=s1w[:, :wsize], start=True, stop=True)
                    ht = htp.tile([128, 128], F16, tag="ht")
                    nc.scalar.activation(out=ht[:, :wsize], in_=ps_h[:, :wsize],
                                         func=AF.Relu, bias=b1_sb[:, j:j + 1])
                    hts.append(ht)
                ps_p = psP.tile([CP, 128], F32, tag="psp", name="ps_p")
                for j in range(NJ):
                    nc.tensor.matmul(out=ps_p[:, :wsize], lhsT=W2_sb[:, j, :],
                                     rhs=hts[j][:, :wsize],
                                     start=(j == 0), stop=(j == NJ - 1))
                ptT = work.tile([CP, 128], F16, tag="ptT")
                nc.any.tensor_copy(out=ptT[:, :wsize], in_=ps_p[:, :wsize])
                ps_t2 = psT2.tile([128, CP], F16, tag="pt2", name="ps_t2")
                nc.tensor.transpose(out=ps_t2[:wsize, :], in_=ptT[:, :wsize],
                                    identity=ident16[:CP, :CP])
                pt = work.tile([128, PP], F16, tag="pt")
                nc.any.tensor_copy(out=pt[:wsize, :CP], in_=ps_t2[:wsize, :])
                nc.vector.memset(pt[:wsize, CP:], 0.0)
                nc.sync.dma_start(out=p_sh[d * 128:d * 128 + wsize],
                                  in_=pt[:wsize])

            nc.gpsimd.collective_compute(
                "AllGather", OP.bypass, replica_groups=groups,
                ins=[p_sh[:]], outs=[p_full[:]])
            tc.strict_bb_all_engine_barrier()

            # ---------------- phase D: logitT += (A @ P).T ----------------
            ps_seg2 = [None]

            def d_block(w, sf, sl, gt, j, sel):
                wsize = min(128, NSH - w * 128)
                if sf:
                    ps_seg2[0] = psSeg.tile([128, 128], F32, tag="seg", name="ps_seg2")
                nc.tensor.matmul(out=ps_seg2[0][:CP, :], lhsT=gt[:, j, :CP],
                                 rhs=sel[:], start=sf, stop=sl)
                if sl:
                    nc.vector.tensor_add(
                        out=logitT[:, w * 128:w * 128 + wsize],
                        in0=logitT[:, w * 128:w * 128 + wsize],
                        in1=ps_seg2[0][:CP, :wsize])

            spmm(p_full, PP, d_block)

            # ---------------- phase E: transpose + write logits ----------------
            for w in range(NW):
                wsize = min(128, NSH - w * 128)
                ps_f = psSeg.tile([128, 128], F32, tag="seg", name="ps_f")
                nc.tensor.transpose(out=ps_f[:wsize, :CP],
                                    in_=logitT[:, w * 128:w * 128 + wsize],
                                    identity=ident[:CP, :CP])
                nc.any.tensor_copy(out=logit_sb[:wsize, w, :],
                                   in_=ps_f[:wsize, :CP])

            nf = NSH // 128
            nc.sync.dma_start(
                out=logits[:nf * 128].rearrange("(d p) c -> p d c", p=128),
                in_=logit_sb[:, :nf, :cfg.NCLS])
            if NSH % 128:
                nc.sync.dma_start(out=logits[nf * 128:],
                                  in_=logit_sb[:NSH % 128, nf, :cfg.NCLS])

    nc.compile()
    return nc


# ----------------------------------------------------------------------------
# Entry point
# ----------------------------------------------------------------------------

def _run(cfg: Cfg, inputs: dict, trace: bool = False):
    in_maps, meta = build_host(cfg, inputs)
    nc = build_program(cfg, meta)
    res = run_bass_kernel_spmd(nc, in_maps, list(range(cfg.NCORES)), trace=trace)
    out = np.concatenate([res.results[k]["logits"] for k in range(cfg.NCORES)], axis=0)
    return out, res


def kernel(**inputs) -> np.ndarray:
    cfg = Cfg()
    out, _ = _run(cfg, inputs, trace=False)
    return out.astype(np.float32)


if __name__ == "__main__":
    # smoke test at reduced scale against a numpy reference
    cfg = Cfg(N=2048, E=32768, NCORES=8, NCHUNK=2)
    rng = np.random.default_rng(0)
    inputs = {
        "feature": rng.standard_normal((cfg.N, cfg.D_IN), dtype=np.float32),
        "conv_w": rng.standard_normal((4, 1, 5), dtype=np.float32) * 0.2,
        "conv_b": np.zeros(4, np.float32),
        "W1": rng.standard_normal((cfg.D_IN, cfg.D_HID), dtype=np.float32) * 0.1,
        "b1": np.zeros(cfg.D_HID, np.float32),
        "W2": rng.standard_normal((cfg.D_HID, cfg.NCLS), dtype=np.float32) * 0.05,
        "b2": np.zeros(cfg.NCLS, np.float32),
        "adj_val": rng.random(cfg.E, dtype=np.float32),
        "edge_row": rng.integers(0, cfg.N, cfg.E).astype(np.int32),
        "edge_col": rng.integers(0, cfg.N, cfg.E).astype(np.int32),
    }
    out, _ = _run(cfg, inputs)

    # numpy reference
    ws = inputs["conv_w"].sum(axis=0).ravel()
    xr = np.zeros((cfg.N, cfg.D_IN), np.float32)
    f = inputs["feature"]
    for k in range(5):
        s = k - 2
        lo, hi = max(0, -s), min(cfg.D_IN, cfg.D_IN - s)
        xr[:, lo:hi] += ws[k] * f[:, lo + s:hi + s]
    xr = np.maximum(xr + inputs["conv_b"].sum(), 0)
    S1 = np.zeros_like(xr)
    np.add.at(S1, inputs["edge_row"],
              inputs["adj_val"][:, None] * xr[inputs["edge_col"]])
    h = np.maximum(S1 @ inputs["W1"] + inputs["b1"], 0)
    P = h @ inputs["W2"]
    Y = np.zeros_like(P)
    np.add.at(Y, inputs["edge_row"], inputs["adj_val"][:, None] * P[inputs["edge_col"]])
    Y += inputs["b2"]
    err = np.abs(out - Y).max() / (np.abs(Y).max() + 1e-30)
    print("rel err:", err)


# revision 8
# speedup vs baseline: 1.5937x; 1.5937x over previous
"""Trainium2 Bass kernel for nn_Brep_Gcn (GCN message passing).

Math (reference):
    x  = relu(sum_ch conv1d(feature))           # conv folds to a banded matmul
    h  = relu(A @ (x W1) + b1) = relu((A @ x) W1 + b1)
    y  = A @ (h W2) + b2

Reordered for memory efficiency:
    S1 = A @ x          (sparse gather + segment-sum, fp16 rows)
    h  = relu(S1 W1 + b1)
    P  = h W2
    y  = A @ P + b2     (sparse gather + segment-sum, fp16 rows)

Distribution: nodes row-sharded across 8 cores; edges partitioned by
destination owner.  The conv prelude is REPLICATED on every core (writing a
private fp16 x_full) which removes the x AllGather entirely; P is exchanged
with one AllGather.  Weights replicated.

Sparse segment-sum on device: edges sorted by (dest-window, src-chunk),
padded to 128-edge blocks.  Per block: dma_gather the 128 source rows (fp16,
256 B rows), build a one-hot selector Sel[e, d] = val[e] * (slot[e] == d)
with one fused tensor_scalar, and matmul on the PE accumulating into PSUM
per (window, chunk) segment.  Gather calls batch 32 blocks (4096 idxs) and
round-robin the 4 SWDGE queues so descriptor generation runs on all four
Q7 core-pairs concurrently.
"""

import math
import os
import sys
from dataclasses import dataclass

import numpy as np

sys.path.insert(0, "/opt/trn_rl_repo")

import concourse.bass as bass
import concourse.tile as tile
from concourse import bacc
from concourse import mybir
from concourse.bass_utils import run_bass_kernel_spmd
from concourse.masks import make_identity

F32 = mybir.dt.float32
F16 = mybir.dt.float16
I16 = mybir.dt.int16
I32 = mybir.dt.int32
AF = mybir.ActivationFunctionType
OP = mybir.AluOpType


@dataclass
class Cfg:
    N: int = 100000
    E: int = 3200000
    D_IN: int = 83
    D_HID: int = 1024
    NCLS: int = 25
    NCORES: int = 8
    NCHUNK: int = 4          # source-index chunks (int16 gather indices)
    XPAD: int = 128          # padded x row, fp16 (256 B)
    PPAD: int = 128          # padded P row, fp16 (256 B)
    CPAD: int = 32           # classes padded (25 -> 32)
    GBLK: int = 32           # 128-edge blocks per dma_gather call (4096 idxs)
    IDXG: int = 4            # gather calls per idx-staging DMA
    NQ: int = 4              # SWDGE queues (round-robin)

    @property
    def NSH(self):
        return self.N // self.NCORES

    @property
    def CHUNK(self):
        return self.N // self.NCHUNK

    @property
    def NW(self):            # dest windows (of 128) per core
        return (self.NSH + 127) // 128

    @property
    def NJ(self):            # hidden dim in 128-blocks
        return self.D_HID // 128


# ----------------------------------------------------------------------------
# Host-side preprocessing
# ----------------------------------------------------------------------------

def _wrap_idx16(idx: np.ndarray) -> np.ndarray:
    """dma_gather index layout: idx i at [i % 16, i // 16], tiled to 128
    partitions (replicated for the 8 Q7 cores)."""
    assert idx.size % 16 == 0
    a = idx.reshape(-1, 16).T.astype(np.int16)       # [16, n/16]
    return np.tile(a, (8, 1))                        # [128, n/16]


def build_host(cfg: Cfg, inputs: dict) -> tuple[list[dict], dict]:
    """Returns (per-core input maps, shared structure metadata)."""
    N, E = cfg.N, cfg.E
    NSH, NW, NCH, CH = cfg.NSH, cfg.NW, cfg.NCHUNK, cfg.CHUNK

    feature = np.asarray(inputs["feature"], np.float32)
    conv_w = np.asarray(inputs["conv_w"], np.float32)
    conv_b = np.asarray(inputs["conv_b"], np.float32)
    W1 = np.asarray(inputs["W1"], np.float32)
    b1 = np.asarray(inputs["b1"], np.float32)
    W2 = np.asarray(inputs["W2"], np.float32)
    b2 = np.asarray(inputs["b2"], np.float32)
    val = np.asarray(inputs["adj_val"], np.float32)
    row = np.asarray(inputs["edge_row"], np.int64)
    col = np.asarray(inputs["edge_col"], np.int64)

    # conv1d(1->4, k=5, pad 2) summed over channels == banded matmul.
    ws = conv_w.sum(axis=0).ravel()                  # [5]
    b0 = float(conv_b.sum())
    C = np.zeros((cfg.D_IN, cfg.XPAD), np.float32)
    for i in range(cfg.D_IN):
        for k in range(5):
            j = i - (k - 2)                          # out[:, j] += ws[k] * in[:, j + k - 2]
            if 0 <= j < cfg.D_IN:
                C[i, j] = ws[k]

    # ---- edge partitioning: by dest core, then (dest-window, src-chunk) ----
    core_of = row // NSH
    per_core = []
    cnt = np.zeros((cfg.NCORES, NW, NCH), np.int64)
    for k in range(cfg.NCORES):
        m = core_of == k
        r, c_, v = row[m] - k * NSH, col[m], val[m]
        w = r >> 7
        ch = c_ // CH
        order = np.lexsort((c_, ch, w))
        r, c_, v, w, ch = r[order], c_[order], v[order], w[order], ch[order]
        key = w * NCH + ch
        cnt[k] = np.bincount(key, minlength=NW * NCH).reshape(NW, NCH)
        per_core.append((r, c_, v, key))

    # uniform block counts across cores (same program on every core)
    M = np.maximum(1, np.ceil(cnt.max(axis=0) / 128).astype(np.int64))  # [NW, NCH]

    # block metadata, chunk-major (same for every core)
    blocks = []      # (w, chunk, seg_first, seg_last)
    calls = []       # (chunk, blk_start, nblk, idx_off16)
    nblk_total = int(M.sum())
    for ch in range(NCH):
        cblks = []
        for w in range(NW):
            for m in range(int(M[w, ch])):
                cblks.append((w, ch, m == 0, m == int(M[w, ch]) - 1))
        s = 0
        while s < len(cblks):
            n = min(cfg.GBLK, len(cblks) - s)
            calls.append([ch, len(blocks) + s, n, 0])
            s += n
        blocks.extend(cblks)
    assert len(blocks) == nblk_total
    off = 0
    for call in calls:
        call[3] = off
        off += call[2] * 128 // 16
    tot16 = off

    # ---- per-core padded edge arrays in block order ----
    in_maps = []
    for k in range(cfg.NCORES):
        r, c_, v, key = per_core[k]
        pos = np.searchsorted(key, np.arange(NW * NCH + 1), side="left")
        idx_pad = np.zeros(nblk_total * 128, np.int16)
        slot_pad = np.zeros(nblk_total * 128, np.float32)
        val_pad = np.zeros(nblk_total * 128, np.float32)
        bi = 0
        for ch in range(NCH):
            for w in range(NW):
                a, b = pos[w * NCH + ch], pos[w * NCH + ch + 1]
                n = b - a
                mb = int(M[w, ch])
                dst = bi * 128
                idx_pad[dst:dst + n] = (c_[a:b] % CH).astype(np.int16)
                slot_pad[dst:dst + n] = (r[a:b] - (w << 7)).astype(np.float32)
                val_pad[dst:dst + n] = v[a:b]
                bi += mb
        assert bi == nblk_total
        idx_arr = np.zeros((128, tot16), np.int16)
        for ch, bs, nb, o16 in calls:
            seg = idx_pad[bs * 128:(bs + nb) * 128]
            idx_arr[:, o16:o16 + nb * 128 // 16] = _wrap_idx16(seg)
        slot_arr = slot_pad.reshape(nblk_total, 128).T.astype(np.float16).copy()
        val_arr = val_pad.reshape(nblk_total, 128).T.astype(np.float16).copy()

        b1c = b1.reshape(cfg.NJ, 128).T.astype(np.float32).copy()  # [128, NJ]
        W2p = np.zeros((cfg.D_HID, cfg.CPAD), np.float16)
        W2p[:, :cfg.NCLS] = W2.astype(np.float16)
        b2c = np.zeros((cfg.CPAD, 1), np.float32)
        b2c[:cfg.NCLS, 0] = b2

        in_maps.append({
            "feature": feature,
            "Cmat": C.astype(np.float16),
            "W1": W1.astype(np.float16),
            "b1c": b1c,
            "W2p": W2p,
            "b2c": b2c,
            "idx_dr": idx_arr,
            "slot_dr": slot_arr,
            "val_dr": val_arr,
        })

    meta = {"blocks": blocks, "calls": calls, "nblk": nblk_total,
            "tot16": tot16, "b0": b0}
    return in_maps, meta


# ----------------------------------------------------------------------------
# Bass program (identical for every core; per-core data comes via inputs)
# ----------------------------------------------------------------------------

def build_program(cfg: Cfg, meta: dict) -> bass.Bass:
    N, NSH, NW, NCH, CH = cfg.N, cfg.NSH, cfg.NW, cfg.NCHUNK, cfg.CHUNK
    NJ, XP, PP, CP = cfg.NJ, cfg.XPAD, cfg.PPAD, cfg.CPAD
    blocks, calls = meta["blocks"], meta["calls"]
    nblk, tot16 = meta["nblk"], meta["tot16"]
    groups = [list(range(cfg.NCORES))]

    nc = bacc.Bacc("TRN2", target_bir_lowering=False, debug=False,
                   num_devices=cfg.NCORES, num_swdge_queues=cfg.NQ)

    feature = nc.declare_dram_parameter("feature", [N, cfg.D_IN], F32, isOutput=False)
    Cmat = nc.declare_dram_parameter("Cmat", [cfg.D_IN, XP], F16, isOutput=False)
    W1 = nc.declare_dram_parameter("W1", [cfg.D_IN, cfg.D_HID], F16, isOutput=False)
    b1c = nc.declare_dram_parameter("b1c", [128, NJ], F32, isOutput=False)
    W2p = nc.declare_dram_parameter("W2p", [cfg.D_HID, CP], F16, isOutput=False)
    b2c = nc.declare_dram_parameter("b2c", [CP, 1], F32, isOutput=False)
    idx_dr = nc.declare_dram_parameter("idx_dr", [128, tot16], I16, isOutput=False)
    slot_dr = nc.declare_dram_parameter("slot_dr", [128, nblk], F16, isOutput=False)
    val_dr = nc.declare_dram_parameter("val_dr", [128, nblk], F16, isOutput=False)
    logits = nc.declare_dram_parameter("logits", [NSH, cfg.NCLS], F32, isOutput=True)

    x_full = nc.dram_tensor("x_full", [N, XP], F16)
    p_sh = nc.dram_tensor("p_sh", [NSH, PP], F16)
    p_full = nc.dram_tensor("p_full", [N, PP], F16, addr_space="Shared")

    with tile.TileContext(nc) as tc:
        with (
            tc.tile_pool(name="singles", bufs=1) as singles,
            tc.tile_pool(name="work", bufs=3) as work,
            tc.tile_pool(name="cvin", bufs=2) as cvin,
            tc.tile_pool(name="cvout", bufs=2) as cvout,
            tc.tile_pool(name="sel", bufs=3) as selp,
            tc.tile_pool(name="gath", bufs=3) as gathp,
            tc.tile_pool(name="ht", bufs=18) as htp,
            tc.tile_pool(name="psA", bufs=4, space="PSUM") as psA,
            tc.tile_pool(name="psSeg", bufs=2, space="PSUM") as psSeg,
            tc.tile_pool(name="psP", bufs=1, space="PSUM") as psP,
            tc.tile_pool(name="psT2", bufs=1, space="PSUM") as psT2,
        ):
            # ---------------- constants ----------------
            C_sb = singles.tile([cfg.D_IN, XP], F16)
            nc.sync.dma_start(out=C_sb[:], in_=Cmat[:])
            W1_sb = singles.tile([cfg.D_IN, cfg.D_HID], F16)
            nc.sync.dma_start(out=W1_sb[:], in_=W1[:])
            b1_sb = singles.tile([128, NJ], F32)
            nc.sync.dma_start(out=b1_sb[:], in_=b1c[:])
            W2_sb = singles.tile([128, NJ, CP], F16)
            nc.sync.dma_start(out=W2_sb[:], in_=W2p.rearrange("(j p) q -> p j q", p=128))
            b2_sb = singles.tile([CP, 1], F32)
            nc.sync.dma_start(out=b2_sb[:], in_=b2c[:])
            slot_sb = singles.tile([128, nblk], F16)
            nc.sync.dma_start(out=slot_sb[:], in_=slot_dr[:])
            val_sb = singles.tile([128, nblk], F16)
            nc.sync.dma_start(out=val_sb[:], in_=val_dr[:])

            b0_sb = singles.tile([128, 1], F32)
            nc.vector.memset(b0_sb[:], meta["b0"])
            ident = singles.tile([128, 128], F32)
            make_identity(nc, ident[:])
            ident16 = singles.tile([128, 128], F16)
            nc.vector.tensor_copy(out=ident16[:], in_=ident[:])
            iota_i = singles.tile([128, 128], I32)
            nc.gpsimd.iota(iota_i[:], pattern=[[1, 128]], base=0, channel_multiplier=0)
            iota16 = singles.tile([128, 128], F16)
            nc.vector.tensor_copy(out=iota16[:], in_=iota_i[:])

            S1T = singles.tile([cfg.D_IN, NW * 128], F32)
            nc.vector.memset(S1T[:], 0.0)
            # logitT init = b2 broadcast along the free dim
            logitT = singles.tile([CP, NW * 128], F32)
            b2_ap = b2_sb[:]
            b2_bc = bass.AP(tensor=b2_ap.tensor, offset=b2_ap.offset,
                            ap=[b2_ap.ap[0], [0, NW * 128]])
            nc.vector.tensor_copy(out=logitT[:], in_=b2_bc)
            logit_sb = singles.tile([128, NW, CP], F32)

            # ---------------- phase A: replicated conv -> x_full (fp16) -----
            CVB = 4                       # 128-row subtiles per load batch
            nfull = N // 128              # 781 full tiles (+ 32-row tail)
            a_tiles = []                  # (row0, rows) per batch
            t = 0
            while t + CVB <= nfull:
                a_tiles.append((t * 128, CVB * 128))
                t += CVB
            while t < nfull:
                a_tiles.append((t * 128, 128))
                t += 1
            if N % 128:
                a_tiles.append((nfull * 128, N % 128))

            for (r0, rows) in a_tiles:
                nb4 = (rows + 127) // 128
                if rows % 128 == 0 and nb4 > 1:
                    ftb = cvin.tile([128, CVB, cfg.D_IN], F32, tag="ftb")
                    nc.sync.dma_start(
                        out=ftb[:, :nb4, :],
                        in_=feature[r0:r0 + rows].rearrange("(a p) d -> p a d", p=128))
                else:
                    ftb = cvin.tile([128, CVB, cfg.D_IN], F32, tag="ftb")
                    nc.sync.dma_start(out=ftb[:rows, 0, :],
                                      in_=feature[r0:r0 + rows])
                xtb = cvout.tile([128, CVB, XP], F16, tag="xtb")
                for a in range(nb4):
                    sub = min(128, rows - a * 128)
                    ps_t = psA.tile([128, 128], F32, tag="pa", name="ps_t")
                    nc.tensor.transpose(out=ps_t[:cfg.D_IN, :sub],
                                        in_=ftb[:sub, a, :],
                                        identity=ident[:sub, :sub])
                    ftT = work.tile([cfg.D_IN, 128], F16, tag="ftT")
                    nc.any.tensor_copy(out=ftT[:, :sub], in_=ps_t[:cfg.D_IN, :sub])
                    ps_x = psA.tile([128, XP], F32, tag="pa", name="ps_x")
                    nc.tensor.matmul(out=ps_x[:sub], lhsT=ftT[:, :sub], rhs=C_sb[:],
                                     start=True, stop=True)
                    nc.scalar.activation(out=xtb[:sub, a, :], in_=ps_x[:sub],
                                         func=AF.Relu, bias=b0_sb[:sub])
                if rows % 128 == 0 and nb4 > 1:
                    nc.sync.dma_start(
                        out=x_full[r0:r0 + rows].rearrange("(a p) d -> p a d", p=128),
                        in_=xtb[:, :nb4, :])
                else:
                    nc.sync.dma_start(out=x_full[r0:r0 + rows], in_=xtb[:rows, 0, :])

            # ---------------- phase B: L1 SpMM  S1T = (A @ x).T ----------------
            def spmm(src, src_elem, out_cb):
                """Shared gather + one-hot-matmul skeleton for phases B/D."""
                idx_t = None
                g0 = 0
                for ci, (ch, bs, nb, o16) in enumerate(calls):
                    if ci % cfg.IDXG == 0:
                        grp = calls[ci:ci + cfg.IDXG]
                        g0 = o16
                        gn = sum(c[2] for c in grp) * 8
                        idx_t = work.tile([128, cfg.GBLK * 8 * cfg.IDXG], I16,
                                          tag="idx")
                        nc.sync.dma_start(out=idx_t[:, :gn],
                                          in_=idx_dr[:, g0:g0 + gn])
                    n16 = nb * 128 // 16
                    gt = gathp.tile([128, cfg.GBLK, src_elem], F16, tag="gt")
                    nc.gpsimd.dma_gather(
                        out_ap=gt[:, :nb, :],
                        in_ap=src[ch * CH:(ch + 1) * CH, :],
                        idxs_ap=idx_t[:, o16 - g0:o16 - g0 + n16],
                        num_idxs=nb * 128, num_idxs_reg=nb * 128,
                        elem_size=src_elem, queue_num=ci % cfg.NQ,
                        single_packet=(nb * 128 <= 1024))
                    sel3 = selp.tile([128, cfg.GBLK, 128], F16, tag="sel3",
                                     name="sel3")
                    i0 = iota16[:]
                    iota_bc = bass.AP(tensor=i0.tensor, offset=i0.offset,
                                      ap=[i0.ap[0], [0, nb], i0.ap[1]])
                    s0 = slot_sb[:, bs:bs + nb]
                    slot_bc = bass.AP(tensor=s0.tensor, offset=s0.offset,
                                      ap=[s0.ap[0], s0.ap[1], [0, 128]])
                    v0 = val_sb[:, bs:bs + nb]
                    val_bc = bass.AP(tensor=v0.tensor, offset=v0.offset,
                                     ap=[v0.ap[0], v0.ap[1], [0, 128]])
                    nc.vector.tensor_tensor(out=sel3[:, :nb, :], in0=iota_bc,
                                            in1=slot_bc, op=OP.is_equal)
                    nc.vector.tensor_tensor(out=sel3[:, :nb, :],
                                            in0=sel3[:, :nb, :],
                                            in1=val_bc, op=OP.mult)
                    for j in range(nb):
                        w, _ch, sf, sl = blocks[bs + j]
                        out_cb(w, sf, sl, gt, j, sel3[:, j, :])

            ps_seg = [None]

            def b_block(w, sf, sl, gt, j, sel):
                wsize = min(128, NSH - w * 128)
                if sf:
                    ps_seg[0] = psSeg.tile([128, 128], F32, tag="seg", name="ps_seg")
                nc.tensor.matmul(out=ps_seg[0][:], lhsT=gt[:, j, :], rhs=sel,
                                 start=sf, stop=sl)
                if sl:
                    nc.vector.tensor_add(
                        out=S1T[:, w * 128:w * 128 + wsize],
                        in0=S1T[:, w * 128:w * 128 + wsize],
                        in1=ps_seg[0][:cfg.D_IN, :wsize])

            spmm(x_full, XP, b_block)

            # ---------------- phase C: h = relu(S1 W1 + b1); PT = (h W2).T ----
            for d in range(NW):
                wsize = min(128, NSH - d * 128)
                s1w = work.tile([cfg.D_IN, 128], F16, tag="s1w")
                nc.any.tensor_copy(out=s1w[:, :wsize],
                                   in_=S1T[:, d * 128:d * 128 + wsize])
                hts = []
                for j in range(NJ):
                    ps_h = psA.tile([128, 128], F32, tag="pa", name="ps_h")
                    nc.tensor.matmul(out=ps_h[:, :wsize],
                                     lhsT=W1_sb[:, j * 128:(j + 1) * 128],
                                     rhs=s1w[:, :wsize], start=True, stop=True)
                    ht = htp.tile([128, 128], F16, tag="ht")
                    nc.scalar.activation(out=ht[:, :wsize], in_=ps_h[:, :wsize],
                                         func=AF.Relu, bias=b1_sb[:, j:j + 1])
                    hts.append(ht)
                ps_p = psP.tile([CP, 128], F32, tag="psp", name="ps_p")
                for j in range(NJ):
                    nc.tensor.matmul(out=ps_p[:, :wsize], lhsT=W2_sb[:, j, :],
                                     rhs=hts[j][:, :wsize],
                                     start=(j == 0), stop=(j == NJ - 1))
                ptT = work.tile([CP, 128], F16, tag="ptT")
                nc.any.tensor_copy(out=ptT[:, :wsize], in_=ps_p[:, :wsize])
                ps_t2 = psT2.tile([128, CP], F16, tag="pt2", name="ps_t2")
                nc.tensor.transpose(out=ps_t2[:wsize, :], in_=ptT[:, :wsize],
                                    identity=ident16[:CP, :CP])
                pt = work.tile([128, PP], F16, tag="pt")
                nc.any.tensor_copy(out=pt[:wsize, :CP], in_=ps_t2[:wsize, :])
                nc.vector.memset(pt[:wsize, CP:], 0.0)
                nc.sync.dma_start(out=p_sh[d * 128:d * 128 + wsize],
                                  in_=pt[:wsize])

            nc.gpsimd.collective_compute(
                "AllGather", OP.bypass, replica_groups=groups,
                ins=[p_sh[:]], outs=[p_full[:]])
            tc.strict_bb_all_engine_barrier()

            # ---------------- phase D: logitT += (A @ P).T ----------------
            ps_seg2 = [None]

            def d_block(w, sf, sl, gt, j, sel):
                wsize = min(128, NSH - w * 128)
                if sf:
                    ps_seg2[0] = psSeg.tile([128, 128], F32, tag="seg", name="ps_seg2")
                nc.tensor.matmul(out=ps_seg2[0][:CP, :], lhsT=gt[:, j, :CP],
                                 rhs=sel, start=sf, stop=sl)
                if sl:
                    nc.vector.tensor_add(
                        out=logitT[:, w * 128:w * 128 + wsize],
                        in0=logitT[:, w * 128:w * 128 + wsize],
                        in1=ps_seg2[0][:CP, :wsize])

            spmm(p_full, PP, d_block)

            # ---------------- phase E: transpose + write logits ----------------
            for w in range(NW):
                wsize = min(128, NSH - w * 128)
                ps_f = psSeg.tile([128, 128], F32, tag="seg", name="ps_f")
                nc.tensor.transpose(out=ps_f[:wsize, :CP],
                                    in_=logitT[:, w * 128:w * 128 + wsize],
                                    identity=ident[:CP, :CP])
                nc.any.tensor_copy(out=logit_sb[:wsize, w, :],
                                   in_=ps_f[:wsize, :CP])

            nf = NSH // 128
            nc.sync.dma_start(
                out=logits[:nf * 128].rearrange("(d p) c -> p d c", p=128),
                in_=logit_sb[:, :nf, :cfg.NCLS])
            if NSH % 128:
                nc.sync.dma_start(out=logits[nf * 128:],
                                  in_=logit_sb[:NSH % 128, nf, :cfg.NCLS])

    nc.compile()
    return nc


# ----------------------------------------------------------------------------
# Entry point
# ----------------------------------------------------------------------------

def _run(cfg: Cfg, inputs: dict, trace: bool = False):
    in_maps, meta = build_host(cfg, inputs)
    nc = build_program(cfg, meta)
    res = run_bass_kernel_spmd(nc, in_maps, list(range(cfg.NCORES)), trace=trace)
    out = np.concatenate([res.results[k]["logits"] for k in range(cfg.NCORES)], axis=0)
    return out, res


def kernel(**inputs) -> np.ndarray:
    cfg = Cfg()
    out, _ = _run(cfg, inputs, trace=False)
    return out.astype(np.float32)


if __name__ == "__main__":
    # smoke test at reduced scale against a numpy reference
    cfg = Cfg(N=2048, E=32768, NCORES=8, NCHUNK=2)
    rng = np.random.default_rng(0)
    inputs = {
        "feature": rng.standard_normal((cfg.N, cfg.D_IN), dtype=np.float32),
        "conv_w": rng.standard_normal((4, 1, 5), dtype=np.float32) * 0.2,
        "conv_b": np.zeros(4, np.float32),
        "W1": rng.standard_normal((cfg.D_IN, cfg.D_HID), dtype=np.float32) * 0.1,
        "b1": np.zeros(cfg.D_HID, np.float32),
        "W2": rng.standard_normal((cfg.D_HID, cfg.NCLS), dtype=np.float32) * 0.05,
        "b2": np.zeros(cfg.NCLS, np.float32),
        "adj_val": rng.random(cfg.E, dtype=np.float32),
        "edge_row": rng.integers(0, cfg.N, cfg.E).astype(np.int32),
        "edge_col": rng.integers(0, cfg.N, cfg.E).astype(np.int32),
    }
    out, _ = _run(cfg, inputs)

    # numpy reference
    ws = inputs["conv_w"].sum(axis=0).ravel()
    xr = np.zeros((cfg.N, cfg.D_IN), np.float32)
    f = inputs["feature"]
    for k in range(5):
        s = k - 2
        lo, hi = max(0, -s), min(cfg.D_IN, cfg.D_IN - s)
        xr[:, lo:hi] += ws[k] * f[:, lo + s:hi + s]
    xr = np.maximum(xr + inputs["conv_b"].sum(), 0)
    S1 = np.zeros_like(xr)
    np.add.at(S1, inputs["edge_row"],
              inputs["adj_val"][:, None] * xr[inputs["edge_col"]])
    h = np.maximum(S1 @ inputs["W1"] + inputs["b1"], 0)
    P = h @ inputs["W2"]
    Y = np.zeros_like(P)
    np.add.at(Y, inputs["edge_row"], inputs["adj_val"][:, None] * P[inputs["edge_col"]])
    Y += inputs["b2"]
    err = np.abs(out - Y).max() / (np.abs(Y).max() + 1e-30)
    print("rel err:", err)
